# revision 1
# baseline (speedup 1.0000x reference)
"""Trainium2 Bass kernel for nn_DecoderLayer_68212670595779.

Head-sharded attention + one 8-rank AllToAll, SPMD over 8 cores. See the
session memory for the full design. Measured: relative (absmax) error
5.419e-3 vs the fp32 reference; 407-410 us/layer (repeat-loop harness with
the collective stubbed), ~430 us single-shot with the real AllToAll.
"""
import sys

sys.path.insert(0, "/opt/trn_rl_repo")

import numpy as np
import ml_dtypes
from contextlib import ExitStack

import concourse.bass as bass
import concourse.mybir as mybir
import concourse.tile as tile
from concourse.vector_clock import ScopedClock
from concourse.bass_utils import run_bass_kernel_spmd

BF16 = ml_dtypes.bfloat16
FP32 = mybir.dt.float32
BF = mybir.dt.bfloat16
AF = mybir.ActivationFunctionType
ALU = mybir.AluOpType
AX = mybir.AxisListType

B, S, D, H, HD, FF, P = 2, 2048, 1024, 16, 64, 4096, 128
NCORES = 8
NU = [4 - t // 4 for t in range(16)]  # active q slots per kv chunk


# ---------------------------------------------------------------------------
# Workaround: this walrus build allows only ONE semaphore wait on a CTRL
# (Drain) instruction; TileContext's final drain carries one wait per busy
# proc. Split the waits across a chain of drains on the same engine.
def _patched_drain_and_barrier(self, tick_clock, wait_clock):
    nc = self.nc
    drain_inst = nc.sync.drain()
    wait_clock.add_sem_waits(
        drain_inst.ins, ScopedClock({None: tick_clock.global_clock})
    )
    si = drain_inst.ins.sync_info
    waits = list(si.on_wait) if si is not None else []
    if len(waits) > 1:
        si.on_wait = waits[:1]
        for w in waits[1:]:
            extra = nc.sync.drain()
            esi = extra.ins.sync_info
            if esi is None:
                extra.ins.sync_info = mybir.SyncInfo(on_wait=[w], on_update=[])
            else:
                esi.on_wait = [w]
    nc.all_engine_barrier()
    assert self.sems is not None
    popped = nc._tile_sem_poison_stack.pop()
    assert popped is self._sem_poison
    nc.clear_and_free_semaphores(list(self.sems.allocated().values()))
    nc.all_engine_barrier()


tile.TileContext._drain_and_barrier = _patched_drain_and_barrier


def _split_multi_waits(nc):
    """Walrus in this container supports a single sem wait per instruction.
    Move extra waits onto dedicated no-op instructions on the same engine,
    inserted immediately before (engine program order preserves semantics)."""
    n_split = 0
    for fn in nc.m.functions:
        for bb in fn.blocks:
            out = []
            for ins in bb.instructions:
                si = ins.sync_info
                waits = list(si.on_wait) if si is not None else []
                if len(waits) > 1:
                    si.on_wait = [waits[-1]]
                    for i, w in enumerate(waits[:-1]):
                        nop = mybir.InstNoOp(
                            name=f"{ins.name}-sw{i}",
                            engine=ins.engine,
                            bass_nofuse=True,
                            sync_info=mybir.SyncInfo(on_wait=[w], on_update=[]),
                        )
                        out.append(nop)
                        n_split += 1
                out.append(ins)
            bb.instructions[:] = out
    return n_split


def _core_plan(k):
    g = k % 4
    return k // 4, g, g  # batch, head-group, token-quarter


def _tri_mask():
    """{0,1}[kv 128, q 128] within-tile causal keep (kv <= q)."""
    a = np.arange(P)[:, None]
    qq = np.arange(P)[None, :]
    return (a <= qq).astype(np.float32).astype(BF16)


def _build_nc(reps=1, ablate=()):
    ablate = set(ablate)
    nc = bass.Bass()

    def din(name, shape, dt=BF):
        return nc.declare_dram_parameter(name, list(shape), dt, isOutput=False)

    xT_d = din("xT", (P, 8, S))
    mask_d = din("mask", (P, P))
    wq_d = din("wq", (P, 8, 256))
    wk_d = din("wk", (P, 8, 256))
    wv_d = din("wv", (P, 8, 256))
    wo_d = din("wo", (P, 8, D))
    w1_d = din("w1", (P, 8, FF))
    w2_d = din("w2", (P, 32, D))
    sel_d = din("sel", (P, 8), FP32)
    bqT_d = din("bqT", (P, 2), FP32)
    bkT_d = din("bkT", (P, 2), FP32)
    bvT_d = din("bvT", (P, 2), FP32)
    boT_d = din("boT", (P, 8), FP32)
    b1T_d = din("b1T", (P, 32), FP32)
    rows_d = din("rows", (3, D), FP32)  # b2 / gamma / beta
    eye_d = din("eye", (P, P), BF)
    out_d = nc.declare_dram_parameter("out", [512, D], FP32, isOutput=True)
    a2ain_d = nc.dram_tensor("a2ain", [2048, 512], BF)
    a2aout_d = nc.dram_tensor("a2aout", [2048, 512], BF)

    with ExitStack() as top:
        tc = top.enter_context(tile.TileContext(nc))

        const = top.enter_context(tc.tile_pool(name="const", bufs=1))
        persist = top.enter_context(tc.tile_pool(name="persist", bufs=1))

        # ---- constants ----
        ones_sb = const.tile([P, P], FP32, tag="ones")
        nc.vector.memset(ones_sb[:], 1.0)
        onesb_sb = const.tile([P, P], BF, tag="onesb")
        nc.vector.memset(onesb_sb[:], 1.0)
        eye_sb = const.tile([P, P], BF, tag="eye")
        nc.sync.dma_start(eye_sb[:], eye_d[:])
        bq_sb = const.tile([P, 2], FP32, tag="bq")
        nc.sync.dma_start(bq_sb[:], bqT_d[:])
        bk_sb = const.tile([P, 2], FP32, tag="bk")
        nc.sync.dma_start(bk_sb[:], bkT_d[:])
        bv_sb = const.tile([P, 2], FP32, tag="bv")
        nc.sync.dma_start(bv_sb[:], bvT_d[:])
        sel_sb = const.tile([P, 8], FP32, tag="sel")
        nc.sync.dma_start(sel_sb[:], sel_d[:])
        bo_sb = const.tile([P, 8], FP32, tag="bo")
        nc.sync.dma_start(bo_sb[:], boT_d[:])
        b1_sb = const.tile([P, 32], FP32, tag="b1")
        nc.sync.dma_start(b1_sb[:], b1T_d[:])
        # b2 at row 0, gamma at row 32, beta at row 64 (matmul rhs base part.)
        rows_sb = const.tile([P, D], FP32, tag="rows")
        nc.sync.dma_start(rows_sb[0:1, :], rows_d[0:1, :])
        nc.sync.dma_start(rows_sb[32:33, :], rows_d[1:2, :])
        nc.sync.dma_start(rows_sb[64:65, :], rows_d[2:3, :])

        attV = persist.tile([P, 2, S], BF, tag="attV")      # local heads
        attVf = persist.tile([P, 8, 512], BF, tag="attVf")  # post-A2A full
        attnTb = persist.tile([P, 8, 512], BF, tag="attnTb")
        if ablate:
            nc.vector.memset(attV[:], 0.25)
            nc.vector.memset(attVf[:], 0.25)
            nc.vector.memset(attnTb[:], 0.25)

        if reps > 1:
            # timing-only variant: repeat the whole body on-device so HW time
            # dominates host/tunnel dispatch overhead
            top.enter_context(tc.For_i(0, reps, 1))

        # =========================== phase 1 ===========================
        with ExitStack() as ph1:
            p1 = ph1.enter_context(tc.tile_pool(name="p1", bufs=1))
            KT = p1.tile([P, 2, S], BF, tag="KT")
            Vp = p1.tile([P, 16, 4 * 65], BF, tag="Vp")
            QT = p1.tile([P, 2, S], BF, tag="QT")

            for t in range(16):
                vv = Vp[:, t, :].rearrange("p (b j) -> p b j", j=65)
                nc.vector.memset(vv[:, :, 64:65], 1.0)

            with ExitStack() as ph1a:
                xp = ph1a.enter_context(tc.tile_pool(name="xp", bufs=1))
                wpool = ph1a.enter_context(tc.tile_pool(name="wqkv", bufs=2))
                pp_mm = ph1a.enter_context(
                    tc.tile_pool(name="ppmm1", bufs=3, space="PSUM")
                )

                xT_sb = xp.tile([P, 8, S], BF, tag="xT")
                nc.sync.dma_start(xT_sb[:], xT_d[:])
                _skip_proj = "compute" in ablate

                # ---- Q/K projections for local 4 heads: [256 dout, 2048] ----
                wq_sb = wpool.tile([P, 8, 256], BF, tag="w")
                nc.sync.dma_start(wq_sb[:], wq_d[:])
                wk_sb = wpool.tile([P, 8, 256], BF, tag="w")
                nc.sync.dma_start(wk_sb[:], wk_d[:])
                for dst, w_sb, b_sb in (
                    (QT, wq_sb, bq_sb),
                    (KT, wk_sb, bk_sb),
                ):
                    for m in range(2 if not _skip_proj else 0):
                        for ng in range(4):
                            ps = pp_mm.tile(
                                [P, 512], FP32, tag="mm",
                                name=f"qk_{m}_{ng}",
                            )
                            for kc in range(8):
                                nc.tensor.matmul(
                                    ps[:],
                                    lhsT=w_sb[:, kc, m * P : (m + 1) * P],
                                    rhs=xT_sb[:, kc, ng * 512 : (ng + 1) * 512],
                                    start=(kc == 0),
                                    stop=(kc == 7),
                                )
                            nc.vector.tensor_scalar_add(
                                dst[:, m, ng * 512 : (ng + 1) * 512],
                                ps[:],
                                b_sb[:, m : m + 1],
                            )

                # ---- V projection (local 4 heads, no bias) ----
                wv_sb = wpool.tile([P, 8, 256], BF, tag="w")
                nc.sync.dma_start(wv_sb[:], wv_d[:])
                for tt in range(16 if not _skip_proj else 0):
                    ps = pp_mm.tile([P, 512], FP32, tag="mm", name=f"v_{tt}")
                    for kc in range(8):
                        nc.tensor.matmul(
                            ps[:, :256],
                            lhsT=xT_sb[:, kc, tt * P : (tt + 1) * P],
                            rhs=wv_sb[:, kc, :],
                            start=(kc == 0),
                            stop=(kc == 7),
                        )
                    dst = Vp[:, tt, :].rearrange("p (b j) -> p b j", j=65)[
                        :, :, 0:64
                    ]
                    nc.vector.tensor_copy(
                        dst, ps[:, :256].rearrange("p (b j) -> p b j", j=64)
                    )

            # ---- attention: 2 local head pairs, contiguous causal ----
            with ExitStack() as ph1b:
                mp = ph1b.enter_context(tc.tile_pool(name="mp", bufs=1))
                ptp = ph1b.enter_context(tc.tile_pool(name="ptp", bufs=6))
                srec = ph1b.enter_context(tc.tile_pool(name="srec", bufs=2))
                stg = ph1b.enter_context(tc.tile_pool(name="stg", bufs=3))
                pp_s = ph1b.enter_context(
                    tc.tile_pool(name="pps", bufs=2, space="PSUM")
                )
                pp_o = ph1b.enter_context(
                    tc.tile_pool(name="ppo", bufs=4, space="PSUM")
                )

                mask_sb = mp.tile([P, P], BF, tag="mask")
                nc.sync.dma_start(mask_sb[:], mask_d[:])

                _n_hc = 0 if ("attn" in ablate or "compute" in ablate) else 2
                for hc in range(_n_hc):
                    for g4 in range(4):
                        o_pair = [
                            pp_o.tile(
                                [65, 512], FP32, tag="o", name=f"o_{hc}_{g4}_{i}"
                            )
                            for i in range(2)
                        ]
                        nt = 4 * g4 + 4
                        for t in range(nt):
                            r = t - 4 * g4
                            qoff = max(r, 0) * P
                            N = 512 - qoff
                            s_pair = pp_s.tile(
                                [P, 1024], FP32, tag="s", name=f"s_{hc}_{g4}_{t}"
                            )
                            for i, hp in enumerate((0, 64)):
                                nc.tensor.matmul(
                                    s_pair[:, i * 512 + qoff : (i + 1) * 512],
                                    lhsT=KT[hp : hp + 64, hc, t * P : (t + 1) * P],
                                    rhs=QT[
                                        hp : hp + 64,
                                        hc,
                                        g4 * 512 + qoff : (g4 + 1) * 512,
                                    ],
                                    start=True,
                                    stop=True,
                                )
                            pt = ptp.tile(
                                [P, 1024], BF, tag="pt", name=f"pt_{hc}_{g4}_{t}"
                            )
                            sv = s_pair[:].rearrange("p (h n) -> p h n", h=2)
                            pv = pt[:].rearrange("p (h n) -> p h n", h=2)
                            nc.scalar.activation(
                                pv[:, :, qoff:],
                                sv[:, :, qoff:],
                                AF.Exp,
                                scale=0.125,
                            )
                            if r >= 0:
                                nc.vector.tensor_tensor(
                                    pv[:, :, qoff : qoff + P],
                                    pv[:, :, qoff : qoff + P],
                                    mask_sb[:, None, :].to_broadcast([P, 2, P]),
                                    ALU.mult,
                                )
                            for i in range(2):
                                h = 2 * hc + i
                                nc.tensor.matmul(
                                    o_pair[i][:, qoff:],
                                    lhsT=Vp[:, t, h * 65 : (h + 1) * 65],
                                    rhs=pt[:, i * 512 + qoff : (i + 1) * 512],
                                    start=(t == 0),
                                    stop=(t == nt - 1),
                                )
                        # normalize into attV[local head, g4 block]
                        for i, hp in enumerate((0, 64)):
                            o_ps = o_pair[i]
                            rec = srec.tile(
                                [P, 512], FP32, tag="rec", name=f"rc_{hc}_{g4}_{i}"
                            )
                            nc.vector.reciprocal(rec[64:65, :], o_ps[64:65, :])
                            recb = srec.tile(
                                [P, 512], BF, tag="recb", name=f"rb_{hc}_{g4}_{i}"
                            )
                            nc.vector.tensor_copy(recb[64:65, :], rec[64:65, :])
                            rb = pp_s.tile(
                                [P, 1024], FP32, tag="s", name=f"rbp_{hc}_{g4}_{i}"
                            )
                            nc.tensor.matmul(
                                rb[0:64, :512],
                                lhsT=onesb_sb[64:65, 0:64],
                                rhs=recb[64:65, :],
                                start=True,
                                stop=True,
                            )
                            rbs = srec.tile(
                                [P, 512], FP32, tag="rbs", name=f"rs_{hc}_{g4}_{i}"
                            )
                            nc.vector.tensor_copy(rbs[0:64, :], rb[0:64, :512])
                            avs = attV[
                                hp : hp + 64, hc, g4 * 512 : (g4 + 1) * 512
                            ]
                            nc.vector.tensor_tensor(
                                avs, o_ps[0:64, :], rbs[0:64, :], ALU.mult
                            )
                            nc.vector.tensor_scalar_add(
                                avs, avs, bv_sb[hp : hp + 64, hc : hc + 1]
                            )

                # ---- stage (sel-zeroed) + AllToAll + recombine ----
                for j in range(8):
                    st = stg.tile([P, 2, 512], BF, tag="st", name=f"st_{j}")
                    nc.vector.tensor_scalar_mul(
                        st[:],
                        attV[:, :, (j % 4) * 512 : (j % 4 + 1) * 512],
                        sel_sb[:, j : j + 1],
                    )
                    nc.sync.dma_start(
                        a2ain_d[j * 256 : (j + 1) * 256, :].rearrange(
                            "(c p) q -> p c q", p=P
                        ),
                        st[:],
                    )
                if reps > 1:
                    # collectives cannot sit inside the timing repeat loop;
                    # substitute an equal-size local DMA (timing builds only)
                    nc.sync.dma_start(a2aout_d[:], a2ain_d[:])
                else:
                    nc.gpsimd.collective_compute(
                        "AllToAll",
                        ALU.bypass,
                        ins=[a2ain_d[:]],
                        outs=[a2aout_d[:]],
                        replica_groups=[[0, 1, 2, 3, 4, 5, 6, 7]],
                    )
                halfA = mp.tile([P, 8, 512], BF, tag="hA")
                nc.sync.dma_start(
                    halfA[:],
                    a2aout_d[0:1024, :].rearrange("(i p) q -> p i q", p=P),
                )
                halfB = mp.tile([P, 8, 512], BF, tag="hB")
                nc.sync.dma_start(
                    halfB[:],
                    a2aout_d[1024:2048, :].rearrange("(i p) q -> p i q", p=P),
                )
                nc.vector.tensor_tensor(attVf[:], halfA[:], halfB[:], ALU.add)

        # =========================== phase 2 ===========================
        with ExitStack() as ph2:
            p2 = ph2.enter_context(tc.tile_pool(name="p2", bufs=1))
            w1p = ph2.enter_context(tc.tile_pool(name="w1p", bufs=2))
            w2p = ph2.enter_context(tc.tile_pool(name="w2p", bufs=2))
            lnp = ph2.enter_context(tc.tile_pool(name="lnp", bufs=2))
            smal = ph2.enter_context(tc.tile_pool(name="smal", bufs=2))
            paux2 = ph2.enter_context(
                tc.tile_pool(name="paux2", bufs=2, space="PSUM")
            )
            pp_mm = ph2.enter_context(
                tc.tile_pool(name="ppmm2", bufs=3, space="PSUM")
            )

            # ---- Wo: attnTb[dout, q] (+bo) bf16 ----
            _skip_ffn = "ffn" in ablate or "compute" in ablate
            wo_sb = p2.tile([P, 8, D], BF, tag="wo")
            nc.sync.dma_start(wo_sb[:], wo_d[:])
            for m in range(8 if not _skip_ffn else 0):
                ps = pp_mm.tile([P, 512], FP32, tag="mm")
                for kc in range(8):
                    nc.tensor.matmul(
                        ps[:],
                        lhsT=wo_sb[:, kc, m * P : (m + 1) * P],
                        rhs=attVf[:, kc, :],
                        start=(kc == 0),
                        stop=(kc == 7),
                    )
                nc.vector.tensor_scalar_add(attnTb[:, m, :], ps[:], bo_sb[:, m : m + 1])

            # ---- broadcast rows b2/gamma/beta -> [128, 1024] fp32 ----
            b2b = p2.tile([P, D], FP32, tag="b2b")
            gb = p2.tile([P, D], FP32, tag="gb")
            bb = p2.tile([P, D], FP32, tag="bb")
            for rp, dst in ((0, b2b), (32, gb), (64, bb)):
                for hf in range(2):
                    psb = paux2.tile([P, 512], FP32, tag="aux")
                    nc.tensor.matmul(
                        psb[:],
                        lhsT=ones_sb[rp : rp + 1, :],
                        rhs=rows_sb[rp : rp + 1, hf * 512 : (hf + 1) * 512],
                        start=True,
                        stop=True,
                    )
                    nc.vector.tensor_copy(dst[:, hf * 512 : (hf + 1) * 512], psb[:])

            # ---- W1 + exact GELU (+b1): hT[f, q] bf16 ----
            hT = p2.tile([P, 32, 512], BF, tag="hT")
            for fg in range(8):
                w1_sb = w1p.tile([P, 8, 512], BF, tag="w1")
                nc.sync.dma_start(w1_sb[:], w1_d[:, :, fg * 512 : (fg + 1) * 512])
                for fs in range(4 if not _skip_ffn else 0):
                    f = fg * 4 + fs
                    ps = pp_mm.tile([P, 512], FP32, tag="mm")
                    for kc in range(8):
                        nc.tensor.matmul(
                            ps[:],
                            lhsT=w1_sb[:, kc, fs * P : (fs + 1) * P],
                            rhs=attnTb[:, kc, :],
                            start=(kc == 0),
                            stop=(kc == 7),
                        )
                    nc.scalar.activation(
                        hT[:, f, :], ps[:], AF.Gelu, bias=b1_sb[:, f : f + 1], scale=1.0
                    )

            # ---- transpose attnTb -> attn_sb[q, dout] fp32 (+b2 folded) ----
            attn_sb = p2.tile([P, 4, D], FP32, tag="attn")
            for m in range(8 if not _skip_ffn else 0):
                for t4 in range(4):
                    pst = paux2.tile([P, 512], BF, tag="auxb")
                    nc.tensor.transpose(
                        pst[:, 0:P], attnTb[:, m, t4 * P : (t4 + 1) * P], eye_sb[:]
                    )
                    nc.vector.tensor_tensor(
                        attn_sb[:, t4, m * P : (m + 1) * P],
                        pst[:, 0:P],
                        b2b[:, m * P : (m + 1) * P],
                        ALU.add,
                    )

            # ---- W2 + residual: y[q, dout] fp32 ----
            y_sb = p2.tile([P, 4, D], FP32, tag="y")
            for ng in range(2):
                # stream W2 in two half-contraction tiles for prefetch overlap
                w2_half = []
                for hf in range(2):
                    w2t = w2p.tile(
                        [P, 16, 512], BF, tag="w2", name=f"w2_{ng}_{hf}"
                    )
                    nc.sync.dma_start(
                        w2t[:],
                        w2_d[:, hf * 16 : (hf + 1) * 16, ng * 512 : (ng + 1) * 512],
                    )
                    w2_half.append(w2t)
                for t4 in range(4 if not _skip_ffn else 0):
                    ps = pp_mm.tile([P, 512], FP32, tag="mm")
                    for fc in range(32):
                        nc.tensor.matmul(
                            ps[:],
                            lhsT=hT[:, fc, t4 * P : (t4 + 1) * P],
                            rhs=w2_half[fc // 16][:, fc % 16, :],
                            start=(fc == 0),
                            stop=(fc == 31),
                        )
                    nc.vector.tensor_tensor(
                        y_sb[:, t4, ng * 512 : (ng + 1) * 512],
                        ps[:],
                        attn_sb[:, t4, ng * 512 : (ng + 1) * 512],
                        ALU.add,
                    )

            # ---- LayerNorm + out ----
            for t4 in range(4 if not _skip_ffn else 0):
                yv = y_sb[:, t4, :]
                s1 = smal.tile([P, 1], FP32, tag="s1")
                nc.vector.reduce_sum(s1[:], yv, axis=AX.X)
                sqo = lnp.tile([P, D], FP32, tag="sc")
                s2 = smal.tile([P, 1], FP32, tag="s2")
                nc.scalar.activation(sqo[:], yv, AF.Square, accum_out=s2[:])
                negmean = smal.tile([P, 1], FP32, tag="nm")
                nc.vector.tensor_scalar_mul(negmean[:], s1[:], -1.0 / D)
                mm2 = smal.tile([P, 1], FP32, tag="mm2")
                nc.vector.tensor_tensor(mm2[:], negmean[:], negmean[:], ALU.mult)
                bap = smal.tile([P, 1], FP32, tag="bap")
                nc.vector.tensor_scalar(bap[:], mm2[:], -1.0, 1e-6, ALU.mult, ALU.add)
                std = smal.tile([P, 1], FP32, tag="std")
                nc.scalar.activation(std[:], s2[:], AF.Sqrt, bias=bap[:], scale=1.0 / D)
                rstd = smal.tile([P, 1], FP32, tag="rstd")
                nc.vector.reciprocal(rstd[:], std[:])
                t1 = lnp.tile([P, D], FP32, tag="sc")
                nc.vector.tensor_scalar(
                    t1[:], yv, negmean[:], rstd[:], ALU.add, ALU.mult
                )
                nc.vector.tensor_tensor(t1[:], t1[:], gb[:], ALU.mult)
                nc.vector.tensor_tensor(t1[:], t1[:], bb[:], ALU.add)
                nc.sync.dma_start(out_d[t4 * P : (t4 + 1) * P, :], t1[:])

    _split_multi_waits(nc)
    return nc


_CACHE = {}


def _get_nc(reps=1, ablate=()):
    key = ("nc", reps, tuple(sorted(ablate)))
    if key not in _CACHE:
        _CACHE[key] = _build_nc(reps, ablate)
    return _CACHE[key]


def _prep_in_maps(x, mask, Wq, bq, Wk, bk, Wv, bv, Wo, bo, W1, b1, W2, b2, gamma, beta):
    x = np.asarray(x, np.float32)

    def chunkT(w, nch):
        return np.ascontiguousarray(
            np.asarray(w, np.float32).astype(BF16).reshape(nch, P, -1).transpose(1, 0, 2)
        )

    wo_h = chunkT(Wo, 8)
    w1_h = chunkT(W1, 8)
    w2_h = chunkT(W2, 32)
    Wq = np.asarray(Wq, np.float32)
    Wk = np.asarray(Wk, np.float32)
    Wv = np.asarray(Wv, np.float32)

    def bT(b, nch):
        return np.ascontiguousarray(np.asarray(b, np.float32).reshape(nch, P).T)

    bo_h = bT(bo, 8)
    b1_h = bT(b1, 32)
    rows_h = np.ascontiguousarray(
        np.stack(
            [
                np.asarray(b2, np.float32),
                np.asarray(gamma, np.float32),
                np.asarray(beta, np.float32),
            ]
        )
    )
    eye_h = np.eye(P, dtype=np.float32).astype(BF16)
    mask_h = _tri_mask()
    bq = np.asarray(bq, np.float32)
    bk = np.asarray(bk, np.float32)
    bv = np.asarray(bv, np.float32)

    in_maps = []
    plans = []
    for k in range(NCORES):
        b, g, c = _core_plan(k)
        xb = x[b]
        xT_h = np.ascontiguousarray(
            xb.T.astype(BF16).reshape(8, P, S).transpose(1, 0, 2)
        )
        hs = slice(g * 256, (g + 1) * 256)
        sel_h = np.zeros((P, 8), np.float32)
        sel_h[:, b * 4 : (b + 1) * 4] = 1.0
        in_maps.append(
            {
                "xT": xT_h,
                "mask": mask_h,
                "sel": sel_h,
                "wq": chunkT(Wq[:, hs], 8),
                "wk": chunkT(Wk[:, hs], 8),
                "wv": chunkT(Wv[:, hs], 8),
                "wo": wo_h,
                "w1": w1_h,
                "w2": w2_h,
                "bqT": bT(bq[hs], 2),
                "bkT": bT(bk[hs], 2),
                "bvT": bT(bv[hs], 2),
                "boT": bo_h,
                "b1T": b1_h,
                "rows": rows_h,
                "eye": eye_h,
            }
        )
        plans.append((b, c))
    return in_maps, plans


def kernel(**inputs):
    in_maps, plans = _prep_in_maps(**inputs)
    nc = _get_nc()
    res = run_bass_kernel_spmd(nc, in_maps, core_ids=list(range(NCORES)))
    out = np.zeros((B, S, D), np.float32)
    for k in range(NCORES):
        b, c = plans[k]
        out[b, c * 512 : (c + 1) * 512] = res.results[k]["out"]
    return out



# revision 39
# speedup vs baseline: 1.2548x; 1.2548x over previous
"""Trainium2 Bass kernel for nn_DecoderLayer_68212670595779.

v2: head-sharded attention + two batch-grouped AllToAlls (hc-split so the
first overlaps the second head-pair's attention), deferred softmax
normalization (denominators ride the A2A in the 65th row of each block),
interleaved QKV projections + attention, full-resident W2, per-t4 LN tail.
SPMD over 8 cores: core k = (batch k//4, head-group k%4, token-quarter k%4).
"""
import sys

sys.path.insert(0, "/opt/trn_rl_repo")

import numpy as np
import ml_dtypes
from contextlib import ExitStack

import concourse.bass as bass
import concourse.mybir as mybir
import concourse.tile as tile
from concourse.vector_clock import ScopedClock
from concourse.bass_utils import run_bass_kernel_spmd

BF16 = ml_dtypes.bfloat16
FP32 = mybir.dt.float32
BF = mybir.dt.bfloat16
AF = mybir.ActivationFunctionType
ALU = mybir.AluOpType
AX = mybir.AxisListType

B, S, D, H, HD, FF, P = 2, 2048, 1024, 16, 64, 4096, 128
NCORES = 8


# ---------------------------------------------------------------------------
# Workaround: this walrus build allows only ONE semaphore wait on a CTRL
# (Drain) instruction; TileContext's final drain carries one wait per busy
# proc. Split the waits across a chain of drains on the same engine.
def _patched_drain_and_barrier(self, tick_clock, wait_clock):
    nc = self.nc
    drain_inst = nc.sync.drain()
    wait_clock.add_sem_waits(
        drain_inst.ins, ScopedClock({None: tick_clock.global_clock})
    )
    si = drain_inst.ins.sync_info
    waits = list(si.on_wait) if si is not None else []
    if len(waits) > 1:
        si.on_wait = waits[:1]
        for w in waits[1:]:
            extra = nc.sync.drain()
            esi = extra.ins.sync_info
            if esi is None:
                extra.ins.sync_info = mybir.SyncInfo(on_wait=[w], on_update=[])
            else:
                esi.on_wait = [w]
    nc.all_engine_barrier()
    assert self.sems is not None
    popped = nc._tile_sem_poison_stack.pop()
    assert popped is self._sem_poison
    nc.clear_and_free_semaphores(list(self.sems.allocated().values()))
    nc.all_engine_barrier()


tile.TileContext._drain_and_barrier = _patched_drain_and_barrier


def _split_multi_waits(nc):
    """Walrus in this container supports a single sem wait per instruction.
    Move extra waits onto dedicated no-op instructions on the same engine,
    inserted immediately before (engine program order preserves semantics)."""
    n_split = 0
    for fn in nc.m.functions:
        for bb in fn.blocks:
            out = []
            for ins in bb.instructions:
                si = ins.sync_info
                waits = list(si.on_wait) if si is not None else []
                if len(waits) > 1:
                    si.on_wait = [waits[-1]]
                    for i, w in enumerate(waits[:-1]):
                        nop = mybir.InstNoOp(
                            name=f"{ins.name}-sw{i}",
                            engine=ins.engine,
                            bass_nofuse=True,
                            sync_info=mybir.SyncInfo(on_wait=[w], on_update=[]),
                        )
                        out.append(nop)
                        n_split += 1
                out.append(ins)
            bb.instructions[:] = out
    return n_split


def _tri_mask():
    """{0,1}[kv 128, q 128] within-tile causal keep (kv <= q)."""
    a = np.arange(P)[:, None]
    qq = np.arange(P)[None, :]
    return (a <= qq).astype(np.float32).astype(BF16)


def _patt():
    """[8, 8*128] bf16 scale-broadcast pattern.

    Row r = 2s+i is 1 in block kc (kc//2 == s) at partitions i*64..i*64+63."""
    m = np.zeros((8, 8 * P), np.float32)
    for kc in range(8):
        s = kc // 2
        for i in range(2):
            m[2 * s + i, kc * P + i * 64 : kc * P + i * 64 + 64] = 1.0
    return m.astype(BF16)


def _build_nc(reps=1, stub_collective=False):
    nc = bass.Bass()

    def din(name, shape, dt=BF):
        return nc.declare_dram_parameter(name, list(shape), dt, isOutput=False)

    xT_d = din("xT", (P, 8, S))
    mask_d = din("mask", (P, P))
    wq_d = din("wq", (P, 8, 256))
    wk_d = din("wk", (P, 8, 256))
    wv_d = din("wv", (P, 8, 256))
    wo_d = din("wo", (P, 8, D))
    w1_d = din("w1", (P, 8, FF))
    w2_d = din("w2", (P, 32, D))
    patt_d = din("patt", (8, 8 * P))
    bqT_d = din("bqT", (P, 2), FP32)
    bkT_d = din("bkT", (P, 2), FP32)
    bvT_d = din("bvT", (P, 8), FP32)
    boT_d = din("boT", (P, 8), FP32)
    b1T_d = din("b1T", (P, 32), FP32)
    rows_d = din("rows", (3, D), FP32)  # b2 / gamma / beta
    eye_d = din("eye", (P, P), BF)
    sel_d = din("sel", (P, 8), FP32)
    out_d = nc.declare_dram_parameter("out", [512, D], FP32, isOutput=True)
    # hc-split A2A buffers: [8 dest cores x (2 i-blocks x 65 rows), 512].
    # Chunks for the other batch's cores are zeroed via sel; the receiver
    # adds the two batch halves (only one is nonzero).
    a2ain = [nc.dram_tensor(f"a2ain{h}", [1040, 512], BF) for h in range(2)]
    a2aout = [nc.dram_tensor(f"a2aout{h}", [1040, 512], BF) for h in range(2)]

    groups = [[0, 1, 2, 3, 4, 5, 6, 7]]

    with ExitStack() as top:
        tc = top.enter_context(tile.TileContext(nc))

        const = top.enter_context(tc.tile_pool(name="const", bufs=1))
        persist = top.enter_context(tc.tile_pool(name="persist", bufs=1))

        # ---- constants (outside the rep loop) ----
        eye_sb = const.tile([P, P], BF, tag="eye")
        nc.sync.dma_start(eye_sb[:], eye_d[:])
        mask_sb = const.tile([P, P], BF, tag="mask")
        nc.sync.dma_start(mask_sb[:], mask_d[:])
        patt_sb = const.tile([8, 8 * P], BF, tag="patt")
        nc.sync.dma_start(patt_sb[:], patt_d[:])
        bq_sb = const.tile([P, 2], FP32, tag="bq")
        nc.sync.dma_start(bq_sb[:], bqT_d[:])
        bk_sb = const.tile([P, 2], FP32, tag="bk")
        nc.sync.dma_start(bk_sb[:], bkT_d[:])
        bv_sb = const.tile([P, 8], FP32, tag="bv")
        nc.sync.dma_start(bv_sb[:], bvT_d[:])
        bo_sb = const.tile([P, 8], FP32, tag="bo")
        nc.sync.dma_start(bo_sb[:], boT_d[:])
        b1_sb = const.tile([P, 32], FP32, tag="b1")
        nc.sync.dma_start(b1_sb[:], b1T_d[:])
        sel_sb = const.tile([P, 8], FP32, tag="sel")
        nc.sync.dma_start(sel_sb[:], sel_d[:])

        # broadcast b2/gamma/beta rows -> [128, 1024] fp32, once
        b2b = const.tile([P, D], FP32, tag="b2b")
        gb = const.tile([P, D], FP32, tag="gb")
        bb = const.tile([P, D], FP32, tag="bb")
        with ExitStack() as pre:
            prep = pre.enter_context(tc.tile_pool(name="prep", bufs=1))
            prepp = pre.enter_context(tc.tile_pool(name="prepp", bufs=2, space="PSUM"))
            rows_sb = prep.tile([P, D], FP32, tag="rows")
            nc.sync.dma_start(rows_sb[0:1, :], rows_d[0:1, :])
            nc.sync.dma_start(rows_sb[32:33, :], rows_d[1:2, :])
            nc.sync.dma_start(rows_sb[64:65, :], rows_d[2:3, :])
            ones1 = prep.tile([P, P], FP32, tag="ones1")
            nc.vector.memset(ones1[:], 1.0)
            for rp, dst in ((0, b2b), (32, gb), (64, bb)):
                for hf in range(2):
                    psb = prepp.tile([P, 512], FP32, tag="aux")
                    nc.tensor.matmul(
                        psb[:],
                        lhsT=ones1[rp : rp + 1, :],
                        rhs=rows_sb[rp : rp + 1, hf * 512 : (hf + 1) * 512],
                        start=True,
                        stop=True,
                    )
                    nc.vector.tensor_copy(dst[:, hf * 512 : (hf + 1) * 512], psb[:])

        attVf = persist.tile([P, 8, 512], BF, tag="attVf")   # post-A2A, normalized
        attnTb = persist.tile([P, 8, 512], BF, tag="attnTb")  # Wo out [dout, q]
        # softmax denominators / reciprocals: [row 2s+i, half h, q]
        den = persist.tile([8, 2, 512], BF, tag="den")
        rec = persist.tile([8, 2, 512], BF, tag="rec")
        # raw A2A payload halves before the batch-halves add
        rawA = persist.tile([P, 4, 512], BF, tag="rawA")
        rawB = persist.tile([P, 4, 512], BF, tag="rawB")
        denraw = persist.tile([8, 2, 512], BF, tag="denraw")

        if reps > 1:
            # timing-only variant: repeat the whole body on-device so HW time
            # dominates host/tunnel dispatch overhead
            top.enter_context(tc.For_i(0, reps, 1))

        def a2a(h):
            if reps > 1 or stub_collective:
                # collectives cannot sit inside the timing repeat loop;
                # substitute an equal-size local DMA (timing builds only)
                nc.sync.dma_start(a2aout[h][:], a2ain[h][:])
            else:
                nc.gpsimd.collective_compute(
                    "AllToAll",
                    ALU.bypass,
                    ins=[a2ain[h][:]],
                    outs=[a2aout[h][:]],
                    replica_groups=groups,
                )

        # ======================= phase 1: QKV + attention =================
        with ExitStack() as ph1:
            p1 = ph1.enter_context(tc.tile_pool(name="p1", bufs=1))
            wp = ph1.enter_context(tc.tile_pool(name="wp", bufs=1))
            stg = ph1.enter_context(tc.tile_pool(name="stg", bufs=3))
            ptp = ph1.enter_context(tc.tile_pool(name="ptp", bufs=3))
            work = ph1.enter_context(tc.tile_pool(name="wkp", bufs=2, space="PSUM"))
            pp_s = ph1.enter_context(tc.tile_pool(name="pps", bufs=2, space="PSUM"))
            pp_o = ph1.enter_context(tc.tile_pool(name="ppo", bufs=2, space="PSUM"))

            wq_sb = wp.tile([P, 8, 256], BF, tag="wq")
            nc.sync.dma_start(wq_sb[:], wq_d[:])
            wk_sb = wp.tile([P, 8, 256], BF, tag="wk")
            nc.sync.dma_start(wk_sb[:], wk_d[:])
            wv_sb = wp.tile([P, 8, 256], BF, tag="wv")
            nc.sync.dma_start(wv_sb[:], wv_d[:])
            xT_sb = p1.tile([P, 8, S], BF, tag="xT")
            nc.sync.dma_start(xT_sb[:], xT_d[:])
            # prefetch Wo during phase 1 (needed right at phase-2 start)
            wo_sb = wp.tile([P, 8, D], BF, tag="wo")
            nc.sync.dma_start(wo_sb[:], wo_d[:])

            KT = p1.tile([P, 2, S], BF, tag="KT")
            QT = p1.tile([P, 2, S], BF, tag="QT")
            Vp = p1.tile([P, 16, 4 * 65], BF, tag="Vp")
            for t in range(16):
                vv = Vp[:, t, :].rearrange("p (b j) -> p b j", j=65)
                nc.vector.memset(vv[:, :, 64:65], 1.0)

            # PE warmup during the xT DMA: keeps the HAM clock-gate open so
            # the projection matmuls start at full rate
            for w in range(20):
                psw = work.tile([P, 512], FP32, tag="wk", name=f"warm{w}")
                nc.tensor.matmul(psw[:, 0:P], lhsT=eye_sb[:], rhs=eye_sb[:],
                                 start=True, stop=True)

            def qk_proj(m):
                for dst, w_sb, b_sb in ((QT, wq_sb, bq_sb), (KT, wk_sb, bk_sb)):
                    for ng in range(4):
                        ps = work.tile([P, 512], FP32, tag="wk",
                                       name=f"qk_{m}_{ng}_{0 if dst is QT else 1}")
                        for kc in range(8):
                            nc.tensor.matmul(
                                ps[:],
                                lhsT=w_sb[:, kc, m * P : (m + 1) * P],
                                rhs=xT_sb[:, kc, ng * 512 : (ng + 1) * 512],
                                start=(kc == 0),
                                stop=(kc == 7),
                            )
                        nc.vector.tensor_scalar_add(
                            dst[:, m, ng * 512 : (ng + 1) * 512],
                            ps[:],
                            b_sb[:, m : m + 1],
                        )

            def v_proj(tt):
                ps = work.tile([P, 512], FP32, tag="wk", name=f"v_{tt}")
                for kc in range(8):
                    nc.tensor.matmul(
                        ps[:, :256],
                        lhsT=xT_sb[:, kc, tt * P : (tt + 1) * P],
                        rhs=wv_sb[:, kc, :],
                        start=(kc == 0),
                        stop=(kc == 7),
                    )
                dst = Vp[:, tt, :].rearrange("p (b j) -> p b j", j=65)[:, :, 0:64]
                nc.vector.tensor_copy(
                    dst, ps[:, :256].rearrange("p (b j) -> p b j", j=64)
                )

            def attn(hc, g4):
                """Attention for head-pair hc, query block g4 (512 q).
                Software-pipelined: scores/exp for tile t+1 overlap the
                attn@V accumulation of tile t. Stages the two [65, 512]
                blocks (64 out rows + denominator row) into a2ain[hc]."""
                o_pair = [
                    pp_o.tile([65, 512], FP32, tag="o", name=f"o_{hc}_{g4}_{i}")
                    for i in range(2)
                ]
                nt = 4 * g4 + 4

                def flush(items):
                    for i, pt, qoff, t in items:
                        h = 2 * hc + i
                        nc.tensor.matmul(
                            o_pair[i][:, qoff:],
                            lhsT=Vp[:, t, h * 65 : (h + 1) * 65],
                            rhs=pt[:, i * 512 + qoff : (i + 1) * 512],
                            start=(t == 0),
                            stop=(t == nt - 1),
                        )

                prev = None
                for t in range(nt):
                    r = t - 4 * g4
                    qoff = max(r, 0) * P
                    s_pair = pp_s.tile([P, 1024], FP32, tag="s",
                                       name=f"s_{hc}_{g4}_{t}")
                    for i, hp in enumerate((0, 64)):
                        nc.tensor.matmul(
                            s_pair[:, i * 512 + qoff : (i + 1) * 512],
                            lhsT=KT[hp : hp + 64, hc, t * P : (t + 1) * P],
                            rhs=QT[hp : hp + 64, hc,
                                   g4 * 512 + qoff : (g4 + 1) * 512],
                            start=True,
                            stop=True,
                        )
                    pt = ptp.tile([P, 1024], BF, tag="pt",
                                  name=f"pt_{hc}_{g4}_{t}")
                    sv = s_pair[:].rearrange("p (h n) -> p h n", h=2)
                    pv = pt[:].rearrange("p (h n) -> p h n", h=2)
                    nc.scalar.activation(
                        pv[:, :, qoff:], sv[:, :, qoff:], AF.Exp, scale=0.125
                    )
                    if r >= 0:
                        nc.vector.tensor_tensor(
                            pv[:, :, qoff : qoff + P],
                            pv[:, :, qoff : qoff + P],
                            mask_sb[:, None, :].to_broadcast([P, 2, P]),
                            ALU.mult,
                        )
                    if prev is not None:
                        flush(prev)
                    prev = [(i, pt, qoff, t) for i in range(2)]
                flush(prev)
                for i in range(2):
                    for jb in range(2):
                        j = jb * 4 + g4
                        st = stg.tile([65, 512], BF, tag="st",
                                      name=f"st_{hc}_{g4}_{i}_{jb}")
                        nc.vector.tensor_scalar_mul(
                            st[:], o_pair[i][:], sel_sb[0:65, j : j + 1]
                        )
                        nc.sync.dma_start(
                            a2ain[hc][j * 130 + i * 65 : j * 130 + (i + 1) * 65, :],
                            st[:],
                        )

            def readback(h):
                """a2aout[h] -> rawA/rawB chunks + denominators (both halves)."""
                # denominator rows first ((r mod 65) == 64): the add/reciprocal
                # + scale broadcasts overlap the remaining chunk readbacks
                dview = a2aout[h][:].rearrange("(a p) q -> a p q", p=65)[:, 64, :]
                nc.sync.dma_start(denraw[:, 0, :], dview[0:8, :])
                nc.sync.dma_start(denraw[:, 1, :], dview[8:16, :])
                nc.vector.tensor_tensor(
                    den[:, h, :], denraw[:, 0, :], denraw[:, 1, :], ALU.add
                )
                with nc.allow_low_precision(reason="softmax denom reciprocal in bf16"):
                    nc.vector.reciprocal(rec[:, h, :], den[:, h, :])
                for s in range(4):
                    for i in range(2):
                        nc.sync.dma_start(
                            rawA[i * 64 : (i + 1) * 64, s, :],
                            a2aout[h][s * 130 + i * 65 : s * 130 + i * 65 + 64, :],
                        )
                        nc.sync.dma_start(
                            rawB[i * 64 : (i + 1) * 64, s, :],
                            a2aout[h][(s + 4) * 130 + i * 65 :
                                      (s + 4) * 130 + i * 65 + 64, :],
                        )

            def normalize(h, pool):
                """attVf[:, kc, :] = (rawA + rawB) * (1/den) + bv for half h."""
                for s in range(4):
                    kc = 2 * s + h
                    sc = pool.tile([P, 512], FP32, tag="wk" if pool is work else "sc",
                                   name=f"sc_{h}_{kc}")
                    nc.tensor.matmul(
                        sc[:],
                        lhsT=patt_sb[0:8, kc * P : (kc + 1) * P],
                        rhs=rec[0:8, h, :],
                        start=True,
                        stop=True,
                    )
                    nc.vector.tensor_tensor(
                        attVf[:, kc, :], rawA[:, s, :], rawB[:, s, :], ALU.add
                    )
                    nc.vector.tensor_tensor(
                        attVf[:, kc, :], attVf[:, kc, :], sc[:], ALU.mult
                    )
                    nc.vector.tensor_scalar_add(
                        attVf[:, kc, :], attVf[:, kc, :], bv_sb[:, kc : kc + 1]
                    )

            # ---- emission: interleave projections with hc=0 attention ----
            qk_proj(0)
            for tt in range(4):
                v_proj(tt)
            attn(0, 0)
            for tt in range(4, 8):
                v_proj(tt)
            attn(0, 1)
            for tt in range(8, 12):
                v_proj(tt)
            qk_proj(1)
            attn(0, 2)
            for tt in range(12, 16):
                v_proj(tt)
            attn(0, 3)
            a2a(0)  # overlaps hc=1 attention
            attn(1, 0)
            readback(0)
            attn(1, 1)
            attn(1, 2)
            normalize(0, work)
            attn(1, 3)
            a2a(1)

        # ======================= phase 2: Wo + FFN + LN ===================
        with ExitStack() as ph2:
            p2 = ph2.enter_context(tc.tile_pool(name="p2", bufs=1))
            w1p = ph2.enter_context(tc.tile_pool(name="w1p", bufs=4))
            lnp = ph2.enter_context(tc.tile_pool(name="lnp", bufs=2))
            smal = ph2.enter_context(tc.tile_pool(name="smal", bufs=2))
            yp = ph2.enter_context(tc.tile_pool(name="yp", bufs=2))
            pp_mm = ph2.enter_context(tc.tile_pool(name="ppmm2", bufs=4, space="PSUM"))
            pp_sc = ph2.enter_context(tc.tile_pool(name="ppsc2", bufs=2, space="PSUM"))
            paux = ph2.enter_context(tc.tile_pool(name="paux2", bufs=2, space="PSUM"))

            # finish the second A2A half: denominators first
            dview = a2aout[1][:].rearrange("(a p) q -> a p q", p=65)[:, 64, :]
            nc.sync.dma_start(denraw[:, 0, :], dview[0:8, :])
            nc.sync.dma_start(denraw[:, 1, :], dview[8:16, :])
            nc.vector.tensor_tensor(
                den[:, 1, :], denraw[:, 0, :], denraw[:, 1, :], ALU.add
            )
            with nc.allow_low_precision(reason="softmax denom reciprocal in bf16"):
                nc.vector.reciprocal(rec[:, 1, :], den[:, 1, :])
            for s in range(4):
                for i in range(2):
                    nc.sync.dma_start(
                        rawA[i * 64 : (i + 1) * 64, s, :],
                        a2aout[1][s * 130 + i * 65 : s * 130 + i * 65 + 64, :],
                    )
                    nc.sync.dma_start(
                        rawB[i * 64 : (i + 1) * 64, s, :],
                        a2aout[1][(s + 4) * 130 + i * 65 :
                                  (s + 4) * 130 + i * 65 + 64, :],
                    )

            # weight stream: w1 fg0-3, then w2 in 8 chunks (bounded
            # head-of-line delay on the shared DMA queue)
            w1_tiles = {}
            for fg in range(4):
                w1t = w1p.tile([P, 8, 512], BF, tag="w1", name=f"w1_{fg}")
                nc.sync.dma_start(w1t[:], w1_d[:, :, fg * 512 : (fg + 1) * 512])
                w1_tiles[fg] = w1t
            w2_sb = p2.tile([P, 32, D], BF, tag="w2")
            for hf in range(8):
                nc.sync.dma_start(
                    w2_sb[:, hf * 4 : (hf + 1) * 4, :],
                    w2_d[:, hf * 4 : (hf + 1) * 4, :],
                )

            # ---- Wo: attnTb[dout, q] (+bo) bf16 ----
            # m0-3: even-kc partials run while the second A2A half lands,
            # then odd-kc normalize, then finish. m4-7: plain 8-kc groups.
            wo_ps = {}
            for m in range(4):
                ps = pp_mm.tile([P, 512], FP32, tag="mm", name=f"wo_{m}")
                for j, kc in enumerate((0, 2, 4, 6)):
                    nc.tensor.matmul(
                        ps[:],
                        lhsT=wo_sb[:, kc, m * P : (m + 1) * P],
                        rhs=attVf[:, kc, :],
                        start=(j == 0),
                        stop=False,
                    )
                wo_ps[m] = ps
            for s in range(4):
                kc = 2 * s + 1
                sc = pp_sc.tile([P, 512], FP32, tag="sc", name=f"sco_{kc}")
                nc.tensor.matmul(
                    sc[:],
                    lhsT=patt_sb[0:8, kc * P : (kc + 1) * P],
                    rhs=rec[0:8, 1, :],
                    start=True,
                    stop=True,
                )
                nc.vector.tensor_tensor(
                    attVf[:, kc, :], rawA[:, s, :], rawB[:, s, :], ALU.add
                )
                nc.vector.tensor_tensor(
                    attVf[:, kc, :], attVf[:, kc, :], sc[:], ALU.mult
                )
                nc.vector.tensor_scalar_add(
                    attVf[:, kc, :], attVf[:, kc, :], bv_sb[:, kc : kc + 1]
                )
            for m in range(4):
                ps = wo_ps[m]
                for j, kc in enumerate((1, 3, 5, 7)):
                    nc.tensor.matmul(
                        ps[:],
                        lhsT=wo_sb[:, kc, m * P : (m + 1) * P],
                        rhs=attVf[:, kc, :],
                        start=False,
                        stop=(j == 3),
                    )
                nc.vector.tensor_scalar_add(
                    attnTb[:, m, :], ps[:], bo_sb[:, m : m + 1]
                )
            for m in range(4, 8):
                ps = pp_mm.tile([P, 512], FP32, tag="mm", name=f"wo_{m}")
                for kc in range(8):
                    nc.tensor.matmul(
                        ps[:],
                        lhsT=wo_sb[:, kc, m * P : (m + 1) * P],
                        rhs=attVf[:, kc, :],
                        start=(kc == 0),
                        stop=(kc == 7),
                    )
                nc.vector.tensor_scalar_add(
                    attnTb[:, m, :], ps[:], bo_sb[:, m : m + 1]
                )

            # ---- transpose attnTb -> attn_sb[q, dout] bf16 (+b2 folded) ----
            attn_sb = p2.tile([P, 4, D], BF, tag="attn")
            for m in range(8):
                for t4 in range(4):
                    pst = paux.tile([P, 512], BF, tag="auxb", name=f"tr_{m}_{t4}")
                    nc.tensor.transpose(
                        pst[:, 0:P], attnTb[:, m, t4 * P : (t4 + 1) * P], eye_sb[:]
                    )
                    nc.vector.tensor_tensor(
                        attn_sb[:, t4, m * P : (m + 1) * P],
                        pst[:, 0:P],
                        b2b[:, m * P : (m + 1) * P],
                        ALU.add,
                    )

            # ---- W1 + exact GELU (+b1): hT[f, q] bf16 ----
            hT = p2.tile([P, 32, 512], BF, tag="hT")
            for fg in range(8):
                if fg not in w1_tiles:
                    w1t = w1p.tile([P, 8, 512], BF, tag="w1", name=f"w1_{fg}")
                    nc.sync.dma_start(
                        w1t[:], w1_d[:, :, fg * 512 : (fg + 1) * 512]
                    )
                    w1_tiles[fg] = w1t
                w1t = w1_tiles[fg]
                for fs in range(4):
                    f = fg * 4 + fs
                    ps = pp_mm.tile([P, 512], FP32, tag="mm", name=f"w1m_{f}")
                    for kc in range(8):
                        nc.tensor.matmul(
                            ps[:],
                            lhsT=w1t[:, kc, fs * P : (fs + 1) * P],
                            rhs=attnTb[:, kc, :],
                            start=(kc == 0),
                            stop=(kc == 7),
                        )
                    nc.scalar.activation(
                        hT[:, f, :], ps[:], AF.Gelu, bias=b1_sb[:, f : f + 1],
                        scale=1.0,
                    )

            # ---- W2 + residual + LayerNorm, per 128-token tile ----
            for t4 in range(4):
                y_sb = yp.tile([P, D], FP32, tag="y", name=f"y_{t4}")
                for ng in range(2):
                    ps = pp_mm.tile([P, 512], FP32, tag="mm", name=f"w2_{t4}_{ng}")
                    for fc in range(32):
                        nc.tensor.matmul(
                            ps[:],
                            lhsT=hT[:, fc, t4 * P : (t4 + 1) * P],
                            rhs=w2_sb[:, fc, ng * 512 : (ng + 1) * 512],
                            start=(fc == 0),
                            stop=(fc == 31),
                        )
                    nc.vector.tensor_tensor(
                        y_sb[:, ng * 512 : (ng + 1) * 512],
                        ps[:],
                        attn_sb[:, t4, ng * 512 : (ng + 1) * 512],
                        ALU.add,
                    )
                yv = y_sb[:]
                s1 = smal.tile([P, 1], FP32, tag="s1", name=f"s1_{t4}")
                nc.vector.reduce_sum(s1[:], yv, axis=AX.X)
                sqo = lnp.tile([P, D], BF, tag="sq", name=f"sq_{t4}")
                s2 = smal.tile([P, 1], FP32, tag="s2", name=f"s2_{t4}")
                nc.scalar.activation(sqo[:], yv, AF.Square, accum_out=s2[:])
                negmean = smal.tile([P, 1], FP32, tag="nm", name=f"nm_{t4}")
                nc.vector.tensor_scalar_mul(negmean[:], s1[:], -1.0 / D)
                mm2 = smal.tile([P, 1], FP32, tag="mm2", name=f"mm2_{t4}")
                nc.vector.tensor_tensor(mm2[:], negmean[:], negmean[:], ALU.mult)
                bap = smal.tile([P, 1], FP32, tag="bap", name=f"bap_{t4}")
                nc.vector.tensor_scalar(bap[:], mm2[:], -1.0, 1e-6, ALU.mult, ALU.add)
                std = smal.tile([P, 1], FP32, tag="std", name=f"std_{t4}")
                nc.scalar.activation(std[:], s2[:], AF.Sqrt, bias=bap[:], scale=1.0 / D)
                rstd = smal.tile([P, 1], FP32, tag="rstd", name=f"rstd_{t4}")
                nc.vector.reciprocal(rstd[:], std[:])
                t1 = lnp.tile([P, D], FP32, tag="t1", name=f"t1_{t4}")
                nc.vector.tensor_scalar(
                    t1[:], yv, negmean[:], rstd[:], ALU.add, ALU.mult
                )
                nc.vector.tensor_tensor(t1[:], t1[:], gb[:], ALU.mult)
                nc.vector.tensor_tensor(t1[:], t1[:], bb[:], ALU.add)
                nc.sync.dma_start(out_d[t4 * P : (t4 + 1) * P, :], t1[:])

    _split_multi_waits(nc)
    return nc


_CACHE = {}


def _get_nc(reps=1, stub_collective=False):
    key = ("nc", reps, stub_collective)
    if key not in _CACHE:
        _CACHE[key] = _build_nc(reps, stub_collective)
    return _CACHE[key]


def _prep_in_maps(x, mask, Wq, bq, Wk, bk, Wv, bv, Wo, bo, W1, b1, W2, b2, gamma, beta):
    x = np.asarray(x, np.float32)

    def chunkT(w, nch):
        return np.ascontiguousarray(
            np.asarray(w, np.float32).astype(BF16).reshape(nch, P, -1).transpose(1, 0, 2)
        )

    wo_h = chunkT(Wo, 8)
    w1_h = chunkT(W1, 8)
    w2_h = chunkT(W2, 32)
    Wq = np.asarray(Wq, np.float32)
    Wk = np.asarray(Wk, np.float32)
    Wv = np.asarray(Wv, np.float32)

    def bT(b, nch):
        return np.ascontiguousarray(np.asarray(b, np.float32).reshape(nch, P).T)

    bo_h = bT(bo, 8)
    b1_h = bT(b1, 32)
    bv_h = bT(bv, 8)
    rows_h = np.ascontiguousarray(
        np.stack(
            [
                np.asarray(b2, np.float32),
                np.asarray(gamma, np.float32),
                np.asarray(beta, np.float32),
            ]
        )
    )
    eye_h = np.eye(P, dtype=np.float32).astype(BF16)
    mask_h = _tri_mask()
    patt_h = _patt()
    bq = np.asarray(bq, np.float32)
    bk = np.asarray(bk, np.float32)

    in_maps = []
    plans = []
    for k in range(NCORES):
        b, g = k // 4, k % 4
        xb = x[b]
        xT_h = np.ascontiguousarray(
            xb.T.astype(BF16).reshape(8, P, S).transpose(1, 0, 2)
        )
        sel_h = np.zeros((P, 8), np.float32)
        sel_h[:, b * 4 : (b + 1) * 4] = 1.0
        hs = slice(g * 256, (g + 1) * 256)
        in_maps.append(
            {
                "xT": xT_h,
                "mask": mask_h,
                "patt": patt_h,
                "sel": sel_h,
                "wq": chunkT(Wq[:, hs], 8),
                "wk": chunkT(Wk[:, hs], 8),
                "wv": chunkT(Wv[:, hs], 8),
                "wo": wo_h,
                "w1": w1_h,
                "w2": w2_h,
                "bqT": bT(bq[hs], 2),
                "bkT": bT(bk[hs], 2),
                "bvT": bv_h,
                "boT": bo_h,
                "b1T": b1_h,
                "rows": rows_h,
                "eye": eye_h,
            }
        )
        plans.append((b, g))
    return in_maps, plans


def kernel(**inputs):
    in_maps, plans = _prep_in_maps(**inputs)
    nc = _get_nc()
    res = run_bass_kernel_spmd(nc, in_maps, core_ids=list(range(NCORES)))
    out = np.zeros((B, S, D), np.float32)
    for k in range(NCORES):
        b, c = plans[k]
        out[b, c * 512 : (c + 1) * 512] = res.results[k]["out"]
    return out
